# revision 7
# baseline (speedup 1.0000x reference)
"""Bass/Tile TRN2 kernel for nn_CRF_78907139162441 (CRF message passing).

Math (per batch b, N=64 nodes, D=64*32*32=65536 features):
  F      = a_inter[b].reshape(N, D)
  G      = F @ F.T                       (Gram; diag(G) = squared norms)
  P      = G / (n_i n_j + 1e-6) * (W + W.T)/2     (symmetric, [N, N])
  e_0    = 0
  e_k[i] = sum_j tanh((u_i + e_{k-1}[j]) / 2) * P[i, j]   (10 iterations)
           (2*sigmoid(x) - 1 == tanh(x/2); the reference's unary term
            broadcast makes the loop state rank-1, carried here as e[N])
  out[b] = u + mean(e_10)

Sharding: pure data parallel, one batch per NeuronCore (8 cores).

Implementation per core:
  - DMA f32 tiles [128, 2048] (partition = (half, i), 8 KB contiguous runs)
  - GPSIMD casts f32 -> bf16
  - PE transposes bf16 [128, 128] blocks (via identity), DVE/ACT copy
    PSUM -> SBUF, PE accumulates the Gram in one PSUM bank as
    out[(h,i),(h',i')]; the two diagonal h-blocks sum to G.
  - tiny [64, 64] epilogue: P, then 10 alternating-orientation tanh
    iterations (odd iters reduce along free dim via tensor_tensor_reduce,
    even iters reduce across partitions via an all-ones stationary matmul,
    so no per-iteration transpose is needed).
"""

import os
import sys

import numpy as np

for _p in ("/opt/trn_rl_repo", "/root/.axon_site/_ro/trn_rl_repo"):
    if os.path.isdir(_p) and _p not in sys.path:
        sys.path.insert(0, _p)

import concourse.bass as bass
import concourse.bacc as bacc
import concourse.mybir as mybir
import concourse.tile as tile
from concourse.bass_utils import run_bass_kernel_spmd

B = 8          # batch / cores
N = 64         # nodes
D = 65536      # features per node
NT = 16        # d-band tiles
TF = 2048      # free elems per partition per tile (d-band = 2*TF = 4096)
SUB = TF // 128  # 128-col transpose blocks per tile
ITERATION = 10

F32 = mybir.dt.float32
BF16 = mybir.dt.bfloat16

_CACHE = {}


def build_nc():
    nc = bacc.Bacc("TRN2", target_bir_lowering=False, debug=False)

    a = nc.dram_tensor("a", [N, D], F32, kind="ExternalInput").ap()
    logits = nc.dram_tensor("logits", [N], F32, kind="ExternalInput").ap()
    w = nc.dram_tensor("w", [N, N], F32, kind="ExternalInput").ap()
    eye = nc.dram_tensor("eye", [128, 128], F32, kind="ExternalInput").ap()
    out = nc.dram_tensor("out", [N], F32, kind="ExternalOutput").ap()

    with tile.TileContext(nc) as tc:
        with (
            tc.tile_pool(name="io", bufs=2) as io,
            tc.tile_pool(name="tts", bufs=4) as ttsp,
            tc.tile_pool(name="small", bufs=1) as sm,
            tc.tile_pool(name="ps_tt", bufs=3, space=bass.MemorySpace.PSUM) as ps_tt,
            tc.tile_pool(name="ps_g", bufs=1, space=bass.MemorySpace.PSUM) as ps_g,
            tc.tile_pool(name="ps_s", bufs=2, space=bass.MemorySpace.PSUM) as ps_s,
        ):
            # ---- constants / small inputs (independent of the big stream) ----
            eye_f = sm.tile([128, 128], F32)
            nc.sync.dma_start(eye_f[:], eye[:])
            eye_b = sm.tile([128, 128], BF16)
            nc.vector.tensor_copy(eye_b[:], eye_f[:])

            w_sb = sm.tile([N, N], F32)
            nc.sync.dma_start(w_sb[:], w[:])

            u_row = sm.tile([1, N], F32)
            nc.sync.dma_start(u_row[:], logits.rearrange("(o x) -> o x", o=1))
            u_col = sm.tile([N, 1], F32)
            nc.sync.dma_start(u_col[:], logits.rearrange("(x o) -> x o", o=1))

            ones_col = sm.tile([N, 1], F32)
            nc.vector.memset(ones_col[:], 1.0)
            ones_row = sm.tile([1, N], F32)
            nc.vector.memset(ones_row[:], 1.0)
            ones_nn = sm.tile([N, N], F32)
            nc.vector.memset(ones_nn[:], 1.0)

            u_half_col = sm.tile([N, 1], F32)
            nc.scalar.mul(u_half_col[:], u_col[:], 0.5)
            u_half_row = sm.tile([1, N], F32)
            nc.scalar.mul(u_half_row[:], u_row[:], 0.5)

            # U_half broadcast: rows all equal u/2 (K=1 ones x u_half_row)
            ubh_ps = ps_s.tile([N, N], F32, tag="ps_small")
            nc.tensor.matmul(ubh_ps[:], ones_row[:], u_half_row[:])
            ubh = sm.tile([N, N], F32)
            nc.vector.tensor_copy(ubh[:], ubh_ps[:])

            # ---- Gram: G accumulated as [128, 128] over 256 blocks ----
            g_ps = ps_g.tile([128, 128], F32)
            a_r = a.rearrange("i (t h f) -> t h i f", t=NT, h=2)
            k = 0
            for t in range(NT):
                ftile = io.tile([128, TF], F32, tag="ftile")
                nc.sync.dma_start(ftile[0:N, :], a_r[t, 0])
                nc.sync.dma_start(ftile[N : 2 * N, :], a_r[t, 1])
                btile = io.tile([128, TF], BF16, tag="btile")
                nc.gpsimd.tensor_copy(btile[:], ftile[:])
                for s in range(SUB):
                    ttp = ps_tt.tile([128, 128], BF16, tag="ttp")
                    nc.tensor.transpose(
                        ttp[:], btile[:, s * 128 : (s + 1) * 128], eye_b[:]
                    )
                    tts = ttsp.tile([128, 128], BF16, tag="tts")
                    if k % 2 == 0:
                        nc.vector.tensor_copy(tts[:], ttp[:])
                    else:
                        nc.scalar.copy(tts[:], ttp[:])
                    nc.tensor.matmul(
                        g_ps[:],
                        tts[:],
                        tts[:],
                        start=(k == 0),
                        stop=(k == NT * SUB - 1),
                    )
                    k += 1

            # G = upper-diag block + lower-diag block
            g_hi = sm.tile([N, N], F32)
            nc.vector.tensor_copy(g_hi[:], g_ps[N : 2 * N, N : 2 * N])
            g_sb = sm.tile([N, N], F32)
            nc.vector.tensor_add(g_sb[:], g_ps[0:N, 0:N], g_hi[:])

            # ---- P = G / (n_i n_j + 1e-6) * (W + W.T)/2 ----
            wt_ps = ps_s.tile([N, N], F32, tag="ps_small")
            nc.tensor.transpose(wt_ps[:], w_sb[:], eye_f[0:N, 0:N])
            wsum = sm.tile([N, N], F32)
            nc.vector.tensor_add(wsum[:], w_sb[:], wt_ps[:])

            gi = sm.tile([N, N], F32)
            nc.vector.tensor_mul(gi[:], g_sb[:], eye_f[0:N, 0:N])
            n2r_ps = ps_s.tile([1, N], F32, tag="ps_small")
            nc.tensor.matmul(n2r_ps[:], ones_col[:], gi[:])
            nrm_row = sm.tile([1, N], F32)
            nc.scalar.sqrt(nrm_row[:], n2r_ps[:])

            outer_ps = ps_s.tile([N, N], F32, tag="ps_small")
            nc.tensor.matmul(outer_ps[:], nrm_row[:], nrm_row[:])
            den = sm.tile([N, N], F32)
            nc.vector.tensor_scalar_add(den[:], outer_ps[:], 1e-6)
            rcp = sm.tile([N, N], F32)
            nc.vector.reciprocal(rcp[:], den[:])

            sim_t = sm.tile([N, N], F32)
            nc.vector.tensor_mul(sim_t[:], g_sb[:], rcp[:])
            p_full = sm.tile([N, N], F32)
            nc.vector.tensor_mul(p_full[:], sim_t[:], wsum[:])
            p_sb = sm.tile([N, N], F32)  # p_sb = P/2 = sim*(W+W.T)/4
            nc.vector.tensor_scalar_mul(p_sb[:], p_full[:], 0.25)

            # ---- 10 alternating iterations, state h = e/2 ----
            hfr = sm.tile([N, N], F32, tag="hfr0")  # rows all = e/2 (init 0)
            nc.vector.memset(hfr[:], 0.0)
            h_col = sm.tile([N, 1], F32)
            q_sb = sm.tile([N, N], F32)
            qp = sm.tile([N, N], F32)
            hfr_src = hfr[:]
            for it in range(1, ITERATION + 1):
                if it % 2 == 1:
                    # Q[i,j] = tanh(u_i/2 + e_j/2); h'_col = sum_j Q*(P/2)
                    nc.scalar.activation(
                        q_sb[:], hfr_src,
                        mybir.ActivationFunctionType.Tanh,
                        bias=u_half_col[:],
                    )
                    nc.vector.tensor_mul(qp[:], q_sb[:], p_sb[:])
                    nc.vector.tensor_reduce(
                        h_col[:], qp[:], mybir.AxisListType.X, mybir.AluOpType.add
                    )
                else:
                    # Qt[j,i] = tanh(u_i/2 + e_j/2); H' = ones @ (Qt*(P/2))
                    nc.scalar.activation(
                        q_sb[:], ubh[:],
                        mybir.ActivationFunctionType.Tanh,
                        bias=h_col[:],
                    )
                    nc.vector.tensor_mul(qp[:], q_sb[:], p_sb[:])
                    hfr_ps = ps_s.tile([N, N], F32, tag="ps_small")
                    nc.tensor.matmul(hfr_ps[:], ones_nn[:], qp[:])
                    hfr_src = hfr_ps[:]

            # ---- out = u + mean(e_10) = u + (2/N) * sum_i hfr[0, i] ----
            h_last = sm.tile([1, N], F32)
            nc.vector.tensor_copy(h_last[:], hfr_src[0:1, :])
            red = sm.tile([1, 1], F32)
            nc.vector.tensor_reduce(
                red[:], h_last[:], mybir.AxisListType.X, mybir.AluOpType.add
            )
            mean_b = sm.tile([1, 1], F32)
            nc.vector.tensor_scalar_mul(mean_b[:], red[:], 2.0 / N)
            out_sb = sm.tile([1, N], F32)
            nc.scalar.activation(
                out_sb[:], u_row[:],
                mybir.ActivationFunctionType.Identity,
                bias=mean_b[:],
            )
            nc.sync.dma_start(out.rearrange("(o x) -> o x", o=1), out_sb[:])

    nc.compile()
    return nc


def _in_maps(inputs):
    a_inter = np.ascontiguousarray(inputs["a_inter"], dtype=np.float32)
    logits = np.ascontiguousarray(inputs["logits"], dtype=np.float32)
    w = np.ascontiguousarray(inputs["W"], dtype=np.float32)[0]
    eye = np.eye(128, dtype=np.float32)
    return [
        {
            "a": a_inter[b].reshape(N, D).copy(),
            "logits": logits[b].copy(),
            "w": w.copy(),
            "eye": eye,
        }
        for b in range(B)
    ]


def kernel(**inputs) -> np.ndarray:
    if "nc" not in _CACHE:
        _CACHE["nc"] = build_nc()
    nc = _CACHE["nc"]
    res = run_bass_kernel_spmd(nc, _in_maps(inputs), core_ids=list(range(B)))
    return np.stack([res.results[b]["out"] for b in range(B)], axis=0)


if __name__ == "__main__":
    rng = np.random.default_rng(0)
    ins = {
        "a_inter": rng.standard_normal((B, N, N, 32, 32), dtype=np.float32),
        "logits": rng.standard_normal((B, N), dtype=np.float32),
        "W": rng.standard_normal((1, N, N), dtype=np.float32),
    }
    print(kernel(**ins).shape)


# revision 10
# speedup vs baseline: 1.4154x; 1.4154x over previous
"""Bass/Tile TRN2 kernel for nn_CRF_78907139162441 (CRF message passing).

Math (per batch b, N=64 nodes, D=64*32*32=65536 features):
  F      = a_inter[b].reshape(N, D)
  G      = F @ F.T                       (Gram; diag(G) = squared norms)
  P      = G / (n_i n_j + 1e-6) * (W + W.T)/2     (symmetric, [N, N])
  e_0    = 0
  e_k[i] = sum_j tanh((u_i + e_{k-1}[j]) / 2) * P[i, j]   (10 iterations)
           (2*sigmoid(x) - 1 == tanh(x/2); the reference's unary term
            broadcast makes the loop state rank-1, carried here as e[N])
  out[b] = u + mean(e_10)

Sharding: pure data parallel, one batch per NeuronCore (8 cores).

Implementation per core:
  - DMA f32 tiles [128, 2048] (partition = (half, i), 8 KB contiguous runs)
  - GPSIMD casts f32 -> bf16
  - PE transposes bf16 [128, 128] blocks (via identity), DVE/ACT copy
    PSUM -> SBUF, PE accumulates the Gram in one PSUM bank as
    out[(h,i),(h',i')]; the two diagonal h-blocks sum to G.
  - tiny [64, 64] epilogue: P, then 10 alternating-orientation tanh
    iterations (odd iters reduce along free dim via tensor_tensor_reduce,
    even iters reduce across partitions via an all-ones stationary matmul,
    so no per-iteration transpose is needed).
"""

import os
import sys

import numpy as np

for _p in ("/opt/trn_rl_repo", "/root/.axon_site/_ro/trn_rl_repo"):
    if os.path.isdir(_p) and _p not in sys.path:
        sys.path.insert(0, _p)

import concourse.bass as bass
import concourse.bacc as bacc
import concourse.mybir as mybir
import concourse.tile as tile
from concourse.bass_utils import run_bass_kernel_spmd

B = 8          # batch / cores
N = 64         # nodes
D = 65536      # features per node
NT = 16        # d-band tiles
TF = 2048      # free elems per partition per tile (d-band = 2*TF = 4096)
SUB = TF // 128  # 128-col transpose blocks per tile
ITERATION = 10

F32 = mybir.dt.float32
BF16 = mybir.dt.bfloat16

_CACHE = {}


def build_nc():
    nc = bacc.Bacc("TRN2", target_bir_lowering=False, debug=False)

    a = nc.dram_tensor("a", [N, D], F32, kind="ExternalInput").ap()
    logits = nc.dram_tensor("logits", [N], F32, kind="ExternalInput").ap()
    w = nc.dram_tensor("w", [N, N], F32, kind="ExternalInput").ap()
    eye = nc.dram_tensor("eye", [128, 128], F32, kind="ExternalInput").ap()
    out = nc.dram_tensor("out", [N], F32, kind="ExternalOutput").ap()

    with tile.TileContext(nc) as tc:
        with (
            tc.tile_pool(name="io", bufs=3) as io,
            tc.tile_pool(name="tts", bufs=4) as ttsp,
            tc.tile_pool(name="small", bufs=1) as sm,
            tc.tile_pool(name="ps_tt", bufs=3, space=bass.MemorySpace.PSUM) as ps_tt,
            tc.tile_pool(name="ps_g", bufs=1, space=bass.MemorySpace.PSUM) as ps_g,
            tc.tile_pool(name="ps_s", bufs=2, space=bass.MemorySpace.PSUM) as ps_s,
        ):
            # ---- constants / small inputs (independent of the big stream) ----
            eye_f = sm.tile([128, 128], F32)
            nc.sync.dma_start(eye_f[:], eye[:])

            w_sb = sm.tile([N, N], F32)
            nc.sync.dma_start(w_sb[:], w[:])

            u_row = sm.tile([1, N], F32)
            nc.sync.dma_start(u_row[:], logits.rearrange("(o x) -> o x", o=1))
            u_col = sm.tile([N, 1], F32)
            nc.sync.dma_start(u_col[:], logits.rearrange("(x o) -> x o", o=1))

            ones_col = sm.tile([N, 1], F32)
            nc.vector.memset(ones_col[:], 1.0)
            ones_row = sm.tile([1, N], F32)
            nc.vector.memset(ones_row[:], 1.0)
            ones_nn = sm.tile([N, N], F32)
            nc.vector.memset(ones_nn[:], 1.0)

            u_half_col = sm.tile([N, 1], F32)
            nc.scalar.mul(u_half_col[:], u_col[:], 0.5)
            u_half_row = sm.tile([1, N], F32)
            nc.scalar.mul(u_half_row[:], u_row[:], 0.5)

            # U_half broadcast: rows all equal u/2 (K=1 ones x u_half_row)
            ubh_ps = ps_s.tile([N, N], F32, tag="ps_small")
            nc.tensor.matmul(ubh_ps[:], ones_row[:], u_half_row[:])
            ubh = sm.tile([N, N], F32)
            nc.vector.tensor_copy(ubh[:], ubh_ps[:])

            # ---- Gram: G accumulated as [128, 128] over 256 blocks ----
            # fp32 PE transposes (4 per PSUM bank group); the mandatory
            # PSUM->SBUF copy does the f32->bf16 cast; bf16 Gram matmuls.
            g_ps = ps_g.tile([128, 128], F32)
            a_r = a.rearrange("i (t h f) -> t h i f", t=NT, h=2)
            GRP = 4
            k = 0
            for t in range(NT):
                ftile = io.tile([128, TF], F32, tag="ftile")
                nc.sync.dma_start(ftile[0:N, :], a_r[t, 0])
                nc.sync.dma_start(ftile[N : 2 * N, :], a_r[t, 1])
                for g in range(SUB // GRP):
                    ttp = ps_tt.tile([128, GRP * 128], F32, tag="ttp")
                    for s4 in range(GRP):
                        s = g * GRP + s4
                        nc.tensor.transpose(
                            ttp[:, s4 * 128 : (s4 + 1) * 128],
                            ftile[:, s * 128 : (s + 1) * 128],
                            eye_f[:],
                        )
                    tts = ttsp.tile([128, GRP * 128], BF16, tag="tts")
                    if g % 2 == 0:
                        nc.vector.tensor_copy(tts[:], ttp[:])
                    else:
                        nc.scalar.copy(tts[:], ttp[:])
                    for s4 in range(GRP):
                        nc.tensor.matmul(
                            g_ps[:],
                            tts[:, s4 * 128 : (s4 + 1) * 128],
                            tts[:, s4 * 128 : (s4 + 1) * 128],
                            start=(k == 0),
                            stop=(k == NT * SUB - 1),
                        )
                        k += 1

            # G = upper-diag block + lower-diag block
            g_hi = sm.tile([N, N], F32)
            nc.vector.tensor_copy(g_hi[:], g_ps[N : 2 * N, N : 2 * N])
            g_sb = sm.tile([N, N], F32)
            nc.vector.tensor_add(g_sb[:], g_ps[0:N, 0:N], g_hi[:])

            # ---- P = G / (n_i n_j + 1e-6) * (W + W.T)/2 ----
            wt_ps = ps_s.tile([N, N], F32, tag="ps_small")
            nc.tensor.transpose(wt_ps[:], w_sb[:], eye_f[0:N, 0:N])
            wsum = sm.tile([N, N], F32)
            nc.vector.tensor_add(wsum[:], w_sb[:], wt_ps[:])

            gi = sm.tile([N, N], F32)
            nc.vector.tensor_mul(gi[:], g_sb[:], eye_f[0:N, 0:N])
            n2r_ps = ps_s.tile([1, N], F32, tag="ps_small")
            nc.tensor.matmul(n2r_ps[:], ones_col[:], gi[:])
            nrm_row = sm.tile([1, N], F32)
            nc.scalar.sqrt(nrm_row[:], n2r_ps[:])

            outer_ps = ps_s.tile([N, N], F32, tag="ps_small")
            nc.tensor.matmul(outer_ps[:], nrm_row[:], nrm_row[:])
            den = sm.tile([N, N], F32)
            nc.vector.tensor_scalar_add(den[:], outer_ps[:], 1e-6)
            rcp = sm.tile([N, N], F32)
            nc.vector.reciprocal(rcp[:], den[:])

            sim_t = sm.tile([N, N], F32)
            nc.vector.tensor_mul(sim_t[:], g_sb[:], rcp[:])
            p_full = sm.tile([N, N], F32)
            nc.vector.tensor_mul(p_full[:], sim_t[:], wsum[:])
            p_sb = sm.tile([N, N], F32)  # p_sb = P/2 = sim*(W+W.T)/4
            nc.vector.tensor_scalar_mul(p_sb[:], p_full[:], 0.25)

            # ---- 10 alternating iterations, state h = e/2 ----
            hfr = sm.tile([N, N], F32, tag="hfr0")  # rows all = e/2 (init 0)
            nc.vector.memset(hfr[:], 0.0)
            h_col = sm.tile([N, 1], F32)
            q_sb = sm.tile([N, N], F32)
            qp = sm.tile([N, N], F32)
            hfr_src = hfr[:]
            for it in range(1, ITERATION + 1):
                if it % 2 == 1:
                    # Q[i,j] = tanh(u_i/2 + e_j/2); h'_col = sum_j Q*(P/2)
                    nc.scalar.activation(
                        q_sb[:], hfr_src,
                        mybir.ActivationFunctionType.Tanh,
                        bias=u_half_col[:],
                    )
                    nc.vector.tensor_mul(qp[:], q_sb[:], p_sb[:])
                    nc.vector.tensor_reduce(
                        h_col[:], qp[:], mybir.AxisListType.X, mybir.AluOpType.add
                    )
                else:
                    # Qt[j,i] = tanh(u_i/2 + e_j/2); H' = ones @ (Qt*(P/2))
                    nc.scalar.activation(
                        q_sb[:], ubh[:],
                        mybir.ActivationFunctionType.Tanh,
                        bias=h_col[:],
                    )
                    nc.vector.tensor_mul(qp[:], q_sb[:], p_sb[:])
                    hfr_ps = ps_s.tile([N, N], F32, tag="ps_small")
                    nc.tensor.matmul(hfr_ps[:], ones_nn[:], qp[:])
                    hfr_src = hfr_ps[:]

            # ---- out = u + mean(e_10) = u + (2/N) * sum_i hfr[0, i] ----
            h_last = sm.tile([1, N], F32)
            nc.vector.tensor_copy(h_last[:], hfr_src[0:1, :])
            red = sm.tile([1, 1], F32)
            nc.vector.tensor_reduce(
                red[:], h_last[:], mybir.AxisListType.X, mybir.AluOpType.add
            )
            mean_b = sm.tile([1, 1], F32)
            nc.vector.tensor_scalar_mul(mean_b[:], red[:], 2.0 / N)
            out_sb = sm.tile([1, N], F32)
            nc.scalar.activation(
                out_sb[:], u_row[:],
                mybir.ActivationFunctionType.Identity,
                bias=mean_b[:],
            )
            nc.sync.dma_start(out.rearrange("(o x) -> o x", o=1), out_sb[:])

    nc.compile()
    return nc


def _in_maps(inputs):
    a_inter = np.ascontiguousarray(inputs["a_inter"], dtype=np.float32)
    logits = np.ascontiguousarray(inputs["logits"], dtype=np.float32)
    w = np.ascontiguousarray(inputs["W"], dtype=np.float32)[0]
    eye = np.eye(128, dtype=np.float32)
    return [
        {
            "a": a_inter[b].reshape(N, D).copy(),
            "logits": logits[b].copy(),
            "w": w.copy(),
            "eye": eye,
        }
        for b in range(B)
    ]


def kernel(**inputs) -> np.ndarray:
    if "nc" not in _CACHE:
        _CACHE["nc"] = build_nc()
    nc = _CACHE["nc"]
    res = run_bass_kernel_spmd(nc, _in_maps(inputs), core_ids=list(range(B)))
    return np.stack([res.results[b]["out"] for b in range(B)], axis=0)


if __name__ == "__main__":
    rng = np.random.default_rng(0)
    ins = {
        "a_inter": rng.standard_normal((B, N, N, 32, 32), dtype=np.float32),
        "logits": rng.standard_normal((B, N), dtype=np.float32),
        "W": rng.standard_normal((1, N, N), dtype=np.float32),
    }
    print(kernel(**ins).shape)
